# revision 2
# baseline (speedup 1.0000x reference)
"""Trainium2 Bass kernel v3 for nn_DenseRMoK — latency + pipelining redesign.

Changes vs v2:
- Stats chain restructured: center first (xsc = x + CM, CM needs only the
  x-sum matmuls), square once (sqc = xsc^2, shared by the var matmuls AND
  the taylor x^2 term via x2n = sqc*A2), so var = E[sqc] comes straight out
  of PSUM into one Ln. The tsm/varsm/sdrsm small-vector ops are gone.
- One broadcast matmul of ln(var+eps); ACT exps with scale -0.5/-1.0/+0.5
  produce the istd plane (A), istd^2 plane (A2) and the stdev row without
  any extra hops.
- x2n = sqc*A2 decouples the wave chain from xn: xn (gate + taylor c1) and
  x2n (wave + taylor c2) are computed in parallel on different engines.
- Gate bias folded into the expg activation's per-partition bias port.
- PSUM re-banked with time-disjoint aliases (stats+gate+prs in one bank,
  A/CM/DA in one bank, all 4 G planes in one bank) so the stats/broadcast
  banks and the wave bank are double-buffered: loop iterations overlap.
- Weight DMA double-buffered (wsb in a bufs=2 pool).
"""

import math
import sys

import numpy as np

if "/opt/trn_rl_repo" not in sys.path:
    sys.path.insert(0, "/opt/trn_rl_repo")

B, L, N, P, E = 32, 512, 64, 96, 4
EPS = 1e-5
BN_EPS = 1e-5
MH = 2.0 / (math.sqrt(3.0) * math.pi**0.25)

NCORES = 8
BPC = B // NCORES
R = BPC * N          # 256 rows per core
PD = 128
NCH = L // PD        # 4 l-chunks
WCOLS = NCH * P      # 384 cols per chunked [L,P] weight
WTOT = 6 * WCOLS + NCH * E  # c10|c20|c11|c21|ww0|ww1|wg
WAUX = WTOT + E * P + E     # + bias rows (partition 0) + gate bias row
SELC = E * P + 1            # sel one-hot rows + gate-bias column

_NC_CACHE = {}

# engine assignment per op family; tuned via TimelineSim
STYLE = dict(
    xsc=("vector", "vector", "gpsimd", "gpsimd"),
    sqc=("vector", "vector", "gpsimd", "act"),
    xn=("vector", "gpsimd", "vector", "gpsimd"),
    x2n=("vector", "gpsimd", "gpsimd", "vector"),
    # scalar_tensor_tensor is DVE-only on HW
    psi=("vector", "vector", "vector", "vector"),
    # NOTE: Pool/GPSIMD cannot access PSUM — any op with a PSUM operand
    # must be on vector (DVE) or act.
    ptc="act",       # taylor PSUM -> SBUF copy
    pwc="act",       # G23 plane PSUM -> SBUF copy
    m23="vector",
    s12="vector",
    outp="vector",
)


def _build_nc(debug=False, loop_n=1, style=None, STAGGER=False, unroll=1,
              big_bufs=2, sm_bufs=2, w_bufs=2):
    from contextlib import nullcontext

    import concourse.tile as tile
    from concourse import bacc, mybir
    from concourse._compat import get_trn_type
    from concourse.hw_specs import get_activation_tables

    st = dict(STYLE)
    if style:
        st.update(style)

    f32 = mybir.dt.float32
    f32r = mybir.dt.float32r
    bf16 = mybir.dt.bfloat16
    AF = mybir.ActivationFunctionType
    OP = mybir.AluOpType

    nc = bacc.Bacc(get_trn_type() or "TRN2", target_bir_lowering=False, debug=debug)

    tables = list(get_activation_tables(nc.m.arch).items())
    actset = next(
        i for i, (_, fs) in enumerate(tables)
        if AF.Exp in fs and AF.Ln in fs and AF.Square in fs
    )

    x_d = nc.dram_tensor("x", [PD, NCH * R], bf16, kind="ExternalInput")
    w_d = nc.dram_tensor("w", [PD, WAUX], bf16, kind="ExternalInput")
    sel_d = nc.dram_tensor("sel", [E, SELC], f32r, kind="ExternalInput")
    out_d = nc.dram_tensor("out", [P, R], bf16, kind="ExternalOutput")

    def eng(name):
        return {"vector": nc.vector, "gpsimd": nc.gpsimd, "act": nc.scalar}[name]

    with tile.TileContext(nc) as tc:
        with (
            tc.tile_pool(name="const", bufs=1) as cp,
            tc.tile_pool(name="wp", bufs=w_bufs) as wp,
            tc.tile_pool(name="big", bufs=big_bufs) as bp,
            tc.tile_pool(name="sm", bufs=sm_bufs) as sp,
            tc.tile_pool(name="psA", bufs=2, space="PSUM") as psA,
            tc.tile_pool(name="psM", bufs=1, space="PSUM") as psM,
        ):
            # ---- one-time constants (outside the timing loop) ----
            onesR = cp.tile([1, R], f32r, tag="onesR")
            onesRb = cp.tile([1, R], bf16, tag="onesRb")
            onesRf = cp.tile([1, R], f32, tag="onesRf")
            invL = cp.tile([PD, 1], bf16, tag="invL")
            invLnB = cp.tile([PD, PD], bf16, tag="invLnB")
            invLnBf = cp.tile([PD, PD], f32, tag="invLnBf")
            tmpf = cp.tile([PD, 1], f32, tag="tmpf")
            ones4 = cp.tile([E, 1], f32r, tag="ones4")
            ones4f = cp.tile([E, 1], f32, tag="ones4f")
            cbias = cp.tile([1, 1], f32, tag="cbias")
            zbias = cp.tile([PD, 1], f32, tag="zbias")
            gb4 = cp.tile([E, 1], f32, tag="gb4")
            nc.gpsimd.memset(onesRf, 1.0)
            nc.vector.tensor_copy(onesR, onesRf)
            nc.vector.tensor_copy(onesRb, onesRf)
            nc.gpsimd.memset(tmpf, 1.0 / L)
            nc.vector.tensor_copy(invL, tmpf)
            nc.gpsimd.memset(invLnBf, -1.0 / L)
            nc.vector.tensor_copy(invLnB, invLnBf)
            nc.gpsimd.memset(ones4f, 1.0)
            nc.vector.tensor_copy(ones4, ones4f)
            nc.gpsimd.memset(cbias, EPS)
            nc.vector.memset(zbias, 0.0)
            ones_r = onesR[:, 0:PD]
            sel = cp.tile([E, SELC], f32r, tag="sel")
            nc.scalar.dma_start(out=sel, in_=sel_d[:])
            # gate bias column as f32 for the ACT bias port
            nc.vector.tensor_copy(gb4, sel[:, E * P : E * P + 1].bitcast(f32))
            ld = mybir.InstLoadActFuncSet(
                name=nc.get_next_instruction_name(), ins=[], outs=[]
            )
            ld.act_func_set_id = actset
            ld.engine = nc.scalar.engine
            nc.scalar.add_instruction(ld)

            outer = (
                tc.For_i(0, loop_n, 1, staggered_reset=STAGGER)
                if loop_n > 1
                else nullcontext()
            )
            with outer:
                for _ in range(unroll):
                    # ---- DMAs ----
                    xs = bp.tile([PD, NCH * R], bf16, tag="xs")
                    wsb = wp.tile([PD, WAUX], bf16, tag="wsb")
                    H = NCH * R // 2
                    nc.sync.dma_start(out=xs[:, 0:H], in_=x_d[:, 0:H])
                    nc.sync.dma_start(out=xs[:, H:], in_=x_d[:, H:])
                    nc.sync.dma_start(out=wsb, in_=w_d[:])
                    aux = wsb[0:1, WTOT : WTOT + E * P]

                    c10 = wsb[:, 0 * WCOLS : 1 * WCOLS]
                    c20 = wsb[:, 1 * WCOLS : 2 * WCOLS]
                    c11 = wsb[:, 2 * WCOLS : 3 * WCOLS]
                    c21 = wsb[:, 3 * WCOLS : 4 * WCOLS]
                    ww0 = wsb[:, 4 * WCOLS : 5 * WCOLS]
                    ww1 = wsb[:, 5 * WCOLS : 6 * WCOLS]
                    wg = wsb[:, 6 * WCOLS : 6 * WCOLS + NCH * E]
                    bmm = [aux[:, e * P : (e + 1) * P] for e in range(E)]

                    # ---- PSUM banks ----
                    bankA = psA.tile([PD, 2 * R], f32, tag="bankA")
                    pacB = psA.tile([PD, 2 * R], f32, tag="pacB")
                    ptP = psM.tile([P, 2 * R], f32, tag="ptP")
                    pG01 = psM.tile([P, 2 * R], f32, tag="pG01")
                    pG23 = psM.tile([P, 2 * R], f32, tag="pG23")
                    pwP = psM.tile([P, 2 * R], f32, tag="pwP")
                    ex2_ps = bankA[0:1, R : 2 * R]
                    pg = bankA[0:E, R : 2 * R]        # after lnv reads ex2
                    prs = bankA[0:1, 0:R]
                    pacA = pacB[:, 0:R]
                    pacCM = pacB[:, R : 2 * R]
                    pt0 = ptP[:, 0:R]
                    pt1 = ptP[:, R:]
                    pw0 = pwP[:, 0:R]
                    pw1 = pwP[:, R:]

                    # ---- SBUF tiles ----
                    xsc = bp.tile([PD, NCH * R], bf16, tag="xsc")
                    sqc = bp.tile([PD, NCH * R], bf16, tag="sqc")
                    xn = bp.tile([PD, NCH * R], bf16, tag="xn")
                    x2n = bp.tile([PD, NCH * R], bf16, tag="x2n")
                    eT = bp.tile([PD, NCH * R], bf16, tag="eT")
                    psib = bp.tile([PD, NCH * R], bf16, tag="psib")
                    lnv = sp.tile([1, R], f32r, tag="lnv")
                    stdr = sp.tile([1, R], f32, tag="stdr")
                    rinv = sp.tile([1, R], f32, tag="rinv")
                    dmrow = sp.tile([1, R], f32r, tag="dmrow")
                    expg = sp.tile([E, R], f32r, tag="expg")
                    CMsb = bp.tile([PD, R], bf16, tag="CMsb")
                    Asb = bp.tile([PD, R], bf16, tag="Asb")
                    A2sb = bp.tile([PD, R], bf16, tag="A2sb")
                    ptsb = bp.tile([P, 2 * R], bf16, tag="ptsb")
                    G23sb = bp.tile([P, 2 * R], bf16, tag="G23sb")
                    DAsb = bp.tile([P, R], bf16, tag="DAsb")
                    m01 = bp.tile([P, 2 * R], bf16, tag="m01")
                    m23 = bp.tile([P, 2 * R], bf16, tag="m23")
                    s1 = bp.tile([P, R], bf16, tag="s1")
                    s2 = bp.tile([P, R], bf16, tag="s2")
                    s12 = bp.tile([P, R], bf16, tag="s12")
                    outp = bp.tile([P, R], bf16, tag="outp")

                    def CS(c):
                        return slice(c * R, (c + 1) * R)

                    def CP(c):
                        return slice(c * P, (c + 1) * P)

                    # ---- stats: -mean broadcast directly from the PE ----
                    for c in range(NCH):
                        nc.tensor.matmul(
                            pacCM, invLnB, xs[:, CS(c)],
                            start=(c == 0), stop=(c == NCH - 1),
                        )
                    nc.scalar.activation(CMsb, pacCM, AF.Copy)

                    # ---- xsc = x + CM ; sqc = xsc^2 ; var matmuls ----
                    for c in range(NCH):
                        e = eng(st["xsc"][c])
                        src = pacCM if c == 0 else CMsb
                        e.tensor_add(xsc[:, CS(c)], xs[:, CS(c)], src)
                    for c in range(NCH):
                        en = st["sqc"][c]
                        if en == "act":
                            nc.scalar.activation(
                                sqc[:, CS(c)], xsc[:, CS(c)], AF.Square, bias=zbias
                            )
                        else:
                            eng(en).tensor_mul(
                                sqc[:, CS(c)], xsc[:, CS(c)], xsc[:, CS(c)]
                            )
                        nc.tensor.matmul(
                            ex2_ps, invL, sqc[:, CS(c)],
                            start=(c == 0), stop=(c == NCH - 1),
                        )

                    # ---- var -> ln -> broadcast -> A (istd), A2 (istd^2) ----
                    nc.scalar.activation(lnv, ex2_ps, AF.Ln, bias=cbias)
                    nc.tensor.matmul(pacA, ones_r, lnv, start=True, stop=True)
                    nc.scalar.activation(Asb, pacA, AF.Exp, bias=zbias, scale=-0.5)
                    nc.scalar.activation(A2sb, pacA, AF.Exp, bias=zbias, scale=-1.0)
                    nc.scalar.activation(stdr, lnv, AF.Exp, bias=zbias[0:1], scale=0.5)

                    # ---- bias matmuls OPEN the accumulation groups (their
                    # operands only need the weight DMA, so they never sit in
                    # the PE queue between expg and the G broadcasts) ----
                    nc.tensor.matmul(pt0, bmm[0], onesRb, start=True, stop=False)
                    nc.tensor.matmul(pt1, bmm[1], onesRb, start=False, stop=False)
                    nc.tensor.matmul(pw0, bmm[2], onesRb, start=True, stop=False)
                    nc.tensor.matmul(pw1, bmm[3], onesRb, start=False, stop=False)

                    # ---- xn / x2n / eT / psi elementwise + gate/taylor
                    # matmuls. Wave matmuls are emitted in a second loop so
                    # they never head-of-line-block the taylor close. ----
                    for c in range(NCH):
                        eng(st["xn"][c]).tensor_mul(
                            xn[:, CS(c)], xsc[:, CS(c)], Asb
                        )
                        eng(st["x2n"][c]).tensor_mul(
                            x2n[:, CS(c)], sqc[:, CS(c)], A2sb
                        )
                        last = c == NCH - 1
                        nc.tensor.matmul(
                            pg, wg[:, c * E : (c + 1) * E], xn[:, CS(c)],
                            start=(c == 0), stop=last,
                        )
                        nc.tensor.matmul(pt0, c10[:, CP(c)], xn[:, CS(c)],
                                         start=False, stop=False)
                        nc.tensor.matmul(pt1, c11[:, CP(c)], xn[:, CS(c)],
                                         start=False, stop=False)
                        if last:
                            nc.scalar.activation(expg, pg, AF.Exp, bias=gb4)
                        nc.tensor.matmul(pt0, c20[:, CP(c)], x2n[:, CS(c)],
                                         start=False, stop=False)
                        nc.tensor.matmul(pt1, c21[:, CP(c)], x2n[:, CS(c)],
                                         start=False, stop=last)
                        nc.scalar.activation(eT[:, CS(c)], x2n[:, CS(c)], AF.Exp,
                                             bias=zbias, scale=-0.5)
                        eng(st["psi"][c]).scalar_tensor_tensor(
                            psib[:, CS(c)], x2n[:, CS(c)], -1.0, eT[:, CS(c)],
                            op0=OP.add, op1=OP.mult,
                        )
                        if last:
                            nc.tensor.matmul(prs, ones4, expg, start=True, stop=True)

                    # taylor PSUM -> SBUF as soon as its group closes
                    if st["ptc"] == "act":
                        nc.scalar.activation(ptsb, ptP, AF.Copy)
                    else:
                        eng(st["ptc"]).tensor_copy(ptsb, ptP)

                    # wave matmuls (terminal chain)
                    for c in range(NCH):
                        nc.tensor.matmul(pw0, ww0[:, CP(c)], psib[:, CS(c)],
                                         start=False, stop=False)
                        nc.tensor.matmul(pw1, ww1[:, CP(c)], psib[:, CS(c)],
                                         start=False, stop=(c == NCH - 1))

                    # ---- G planes bcast raw scores immediately; the denorm
                    # scale runs in parallel and applies once at the end ----
                    for e in range(2):
                        nc.tensor.matmul(
                            pG01[:, e * R : (e + 1) * R],
                            sel[:, e * P : (e + 1) * P],
                            expg, start=True, stop=True,
                        )
                        nc.tensor.matmul(
                            pG23[:, e * R : (e + 1) * R],
                            sel[:, (e + 2) * P : (e + 3) * P],
                            expg, start=True, stop=True,
                        )
                    # wave-side score plane to SBUF (overlaps the wave tail)
                    if st["pwc"] == "act":
                        nc.scalar.activation(G23sb, pG23, AF.Copy)
                    else:
                        eng(st["pwc"]).tensor_copy(G23sb, pG23)
                    # denorm row: stdev/(sum expg), broadcast over P partitions
                    nc.vector.reciprocal(rinv, prs)
                    nc.vector.tensor_mul(dmrow, stdr, rinv)
                    pdadc = pacB[0:P, 0:R]   # pacA region, free after Asb/A2sb
                    nc.tensor.matmul(pdadc, ones_r[:, 0:P], dmrow, start=True, stop=True)
                    nc.scalar.activation(DAsb, pdadc, AF.Copy)

                    # ---- mixture + denorm: taylor side = SBUF copy x PSUM
                    # plane; wave side = PSUM accum x SBUF plane ----
                    nc.vector.tensor_mul(m01, ptsb, pG01)
                    nc.vector.tensor_add(s1, m01[:, 0:R], m01[:, R:])
                    eng(st["m23"]).tensor_mul(m23, pwP, G23sb)
                    nc.vector.tensor_add(s2, m23[:, 0:R], m23[:, R:])
                    eng(st["s12"]).tensor_add(s12, s1, s2)
                    nc.vector.tensor_mul(s12, s12, DAsb)
                    eng(st["outp"]).tensor_sub(outp, s12, CMsb[0:P])
                    if st.get("out_ring", "scalar") == "scalar":
                        nc.scalar.dma_start(out=out_d[:], in_=outp)
                    else:
                        nc.sync.dma_start(out=out_d[:], in_=outp)

    nc.compile()
    return nc


def _chunked(wT):
    """[L, M] -> [128, NCH*M], column block c holds rows l = c*128..(c+1)*128."""
    Lx, M = wT.shape
    return np.ascontiguousarray(
        wT.reshape(NCH, PD, M).transpose(1, 0, 2).reshape(PD, NCH * M)
    )


def _host_prep(inputs):
    import ml_dtypes

    f = np.float32
    bf = ml_dtypes.bfloat16
    g = {k: np.asarray(v, f) for k, v in inputs.items()}

    bn_scale = MH / math.sqrt(1.0 + BN_EPS)
    wparts = [
        _chunked(np.ascontiguousarray(g["t0_coeffs"][:, :, 1].T)),
        _chunked(np.ascontiguousarray(g["t0_coeffs"][:, :, 2].T)),
        _chunked(np.ascontiguousarray(g["t1_coeffs"][:, :, 1].T)),
        _chunked(np.ascontiguousarray(g["t1_coeffs"][:, :, 2].T)),
        _chunked(
            np.ascontiguousarray((g["w0_ww"] * g["w0_gamma"][:, None] * bn_scale).T)
        ),
        _chunked(
            np.ascontiguousarray((g["w1_ww"] * g["w1_gamma"][:, None] * bn_scale).T)
        ),
        _chunked(np.ascontiguousarray(g["gate_w"].T)),
    ]
    w_h = np.concatenate(wparts, axis=1).astype(bf)
    assert w_h.shape == (PD, WTOT)

    aux_h = np.zeros((1, E * P + E), f)
    aux_h[0, 0:P] = (
        g["t0_coeffs"][:, :, 0].sum(axis=1, dtype=np.float64) + g["t0_bias"][0]
    ).astype(f)
    aux_h[0, P : 2 * P] = (
        g["t1_coeffs"][:, :, 0].sum(axis=1, dtype=np.float64) + g["t1_bias"][0]
    ).astype(f)
    aux_h[0, 2 * P : 3 * P] = g["w0_beta"]
    aux_h[0, 3 * P : 4 * P] = g["w1_beta"]
    wa_h = np.zeros((PD, E * P + E), f)
    wa_h[0, :] = aux_h[0]
    w_h = np.concatenate([w_h, wa_h.astype(bf)], axis=1)
    assert w_h.shape == (PD, WAUX)

    sel_h = np.zeros((E, SELC), f)
    for e in range(E):
        sel_h[e, e * P : (e + 1) * P] = 1.0
    sel_h[:, E * P] = g["gate_b"] + np.float32(math.log1p(EPS))
    common = {"w": w_h, "sel": sel_h}

    x = g["x"]
    xcores = []
    for i in range(NCORES):
        xc = x[i * BPC : (i + 1) * BPC]  # [BPC, L, N]
        xcores.append(
            np.ascontiguousarray(
                xc.reshape(BPC, NCH, PD, N).transpose(2, 1, 0, 3).reshape(PD, NCH * R)
            ).astype(bf)
        )
    return common, xcores


def _fast_ok(inputs):
    try:
        return (
            np.all(np.asarray(inputs["w0_scale"]) == 1.0)
            and np.all(np.asarray(inputs["w1_scale"]) == 1.0)
            and np.all(np.asarray(inputs["w0_trans"]) == 0.0)
            and np.all(np.asarray(inputs["w1_trans"]) == 0.0)
            and np.all(np.asarray(inputs["rev_w"]) == 1.0)
            and np.all(np.asarray(inputs["rev_b"]) == 0.0)
        )
    except Exception:
        return False


def _numpy_ref(inputs):
    """Exact general fallback (host numpy), mirrors the reference module."""
    g = {k: np.asarray(v, np.float32) for k, v in inputs.items()}
    x = g["x"]
    mean = x.mean(axis=1, keepdims=True)
    stdev = np.sqrt(x.var(axis=1, keepdims=True) + np.float32(EPS))
    xn = (x - mean) / stdev * g["rev_w"] + g["rev_b"]
    xf = xn.transpose(0, 2, 1).reshape(B * N, L)
    logits = xf @ g["gate_w"].T + g["gate_b"]
    logits -= logits.max(axis=-1, keepdims=True)
    elg = np.exp(logits)
    score = elg / elg.sum(axis=-1, keepdims=True)

    def taylor(c, b):
        y = np.full((B * N, P), c[:, :, 0].sum(axis=1), np.float32)
        y += xf @ c[:, :, 1].T + (xf * xf) @ c[:, :, 2].T
        return y + b

    def wave(s, t, w, gam, bet):
        y = np.empty((B * N, P), np.float32)
        for i0 in range(0, B * N, 128):
            xs = (xf[i0 : i0 + 128, None, :] - t[None]) / s[None]
            x2 = xs * xs
            psi = np.float32(MH) * (x2 - 1.0) * np.exp(-0.5 * x2)
            y[i0 : i0 + 128] = np.einsum("bpl,pl->bp", psi, w)
        return (y / np.sqrt(np.float32(1.0 + BN_EPS))) * gam + bet

    eo = np.stack(
        [
            taylor(g["t0_coeffs"], g["t0_bias"][0]),
            taylor(g["t1_coeffs"], g["t1_bias"][0]),
            wave(g["w0_scale"], g["w0_trans"], g["w0_ww"], g["w0_gamma"], g["w0_beta"]),
            wave(g["w1_scale"], g["w1_trans"], g["w1_ww"], g["w1_gamma"], g["w1_beta"]),
        ],
        axis=-1,
    )
    pred = np.einsum("bpE,bE->bp", eo, score)
    pred = pred.reshape(B, N, P).transpose(0, 2, 1)
    out = ((pred - g["rev_b"]) / (g["rev_w"] + np.float32(EPS))) * stdev + mean
    return out.astype(np.float32)


def run(inputs, trace=False):
    """Run the Bass kernel on 8 cores. Returns (out [B,P,N], exec_time_ns|None)."""
    from concourse.bass_utils import run_bass_kernel_spmd

    if "nc" not in _NC_CACHE:
        _NC_CACHE["nc"] = _build_nc()
    nc = _NC_CACHE["nc"]
    common, xcores = _host_prep(inputs)
    in_maps = [dict(common, x=xcores[i]) for i in range(NCORES)]
    try:
        res = run_bass_kernel_spmd(nc, in_maps, list(range(NCORES)), trace=trace)
    except ModuleNotFoundError:
        res = run_bass_kernel_spmd(nc, in_maps, list(range(NCORES)), trace=False)
    out = np.empty((B, P, N), np.float32)
    for i in range(NCORES):
        o = np.asarray(res.results[i]["out"]).astype(np.float32).reshape(P, BPC, N)
        out[i * BPC : (i + 1) * BPC] = o.transpose(1, 0, 2)
    return out, res.exec_time_ns


def kernel(**inputs):
    if not _fast_ok(inputs):
        return _numpy_ref(inputs)
    out, _ = run(inputs)
    return out


# revision 7
# speedup vs baseline: 1.0260x; 1.0260x over previous
"""Trainium2 Bass kernel v3 for nn_DenseRMoK — latency + pipelining redesign.

Changes vs v2:
- Stats chain restructured: center first (xsc = x + CM, CM needs only the
  x-sum matmuls), square once (sqc = xsc^2, shared by the var matmuls AND
  the taylor x^2 term via x2n = sqc*A2), so var = E[sqc] comes straight out
  of PSUM into one Ln. The tsm/varsm/sdrsm small-vector ops are gone.
- One broadcast matmul of ln(var+eps); ACT exps with scale -0.5/-1.0/+0.5
  produce the istd plane (A), istd^2 plane (A2) and the stdev row without
  any extra hops.
- x2n = sqc*A2 decouples the wave chain from xn: xn (gate + taylor c1) and
  x2n (wave + taylor c2) are computed in parallel on different engines.
- Gate bias folded into the expg activation's per-partition bias port.
- PSUM re-banked with time-disjoint aliases (stats+gate+prs in one bank,
  A/CM/DA in one bank, all 4 G planes in one bank) so the stats/broadcast
  banks and the wave bank are double-buffered: loop iterations overlap.
- Weight DMA double-buffered (wsb in a bufs=2 pool).
"""

import math
import sys

import numpy as np

if "/opt/trn_rl_repo" not in sys.path:
    sys.path.insert(0, "/opt/trn_rl_repo")

B, L, N, P, E = 32, 512, 64, 96, 4
EPS = 1e-5
BN_EPS = 1e-5
MH = 2.0 / (math.sqrt(3.0) * math.pi**0.25)

NCORES = 8
BPC = B // NCORES
R = BPC * N          # 256 rows per core
PD = 128
NCH = L // PD        # 4 l-chunks
WCOLS = NCH * P      # 384 cols per chunked [L,P] weight
WTOT = 6 * WCOLS + NCH * E  # c10|c20|c11|c21|ww0|ww1|wg
WAUX = WTOT + E * P + E     # + bias rows (partition 0) + gate bias row
SELC = E * P + 1            # sel one-hot rows + gate-bias column

_NC_CACHE = {}

# engine assignment per op family; tuned via TimelineSim
STYLE = dict(
    xsc=("vector", "vector", "gpsimd", "gpsimd"),
    sqc=("vector", "vector", "gpsimd", "act"),
    xn=("vector", "gpsimd", "vector", "gpsimd"),
    x2n=("vector", "gpsimd", "gpsimd", "vector"),
    # scalar_tensor_tensor is DVE-only on HW
    psi=("vector", "vector", "vector", "vector"),
    # NOTE: Pool/GPSIMD cannot access PSUM — any op with a PSUM operand
    # must be on vector (DVE) or act.
    ptc="act",       # taylor PSUM -> SBUF copy
    pwc="act",       # G23 plane PSUM -> SBUF copy
    m23="vector",
    s12="vector",
    outp="vector",
)


def _build_nc(debug=False, loop_n=1, style=None, STAGGER=False, unroll=1,
              big_bufs=2, sm_bufs=2, w_bufs=3):
    from contextlib import nullcontext

    import concourse.tile as tile
    from concourse import bacc, mybir
    from concourse._compat import get_trn_type
    from concourse.hw_specs import get_activation_tables

    st = dict(STYLE)
    if style:
        st.update(style)

    f32 = mybir.dt.float32
    f32r = mybir.dt.float32r
    bf16 = mybir.dt.bfloat16
    AF = mybir.ActivationFunctionType
    OP = mybir.AluOpType

    nc = bacc.Bacc(get_trn_type() or "TRN2", target_bir_lowering=False, debug=debug)

    tables = list(get_activation_tables(nc.m.arch).items())
    actset = next(
        i for i, (_, fs) in enumerate(tables)
        if AF.Exp in fs and AF.Ln in fs and AF.Square in fs
    )

    x_d = nc.dram_tensor("x", [PD, NCH * R], bf16, kind="ExternalInput")
    w_d = nc.dram_tensor("w", [PD, WAUX], bf16, kind="ExternalInput")
    sel_d = nc.dram_tensor("sel", [E, SELC], f32r, kind="ExternalInput")
    out_d = nc.dram_tensor("out", [P, R], bf16, kind="ExternalOutput")

    def eng(name):
        return {"vector": nc.vector, "gpsimd": nc.gpsimd, "act": nc.scalar}[name]

    with tile.TileContext(nc) as tc:
        with (
            tc.tile_pool(name="const", bufs=1) as cp,
            tc.tile_pool(name="wp", bufs=w_bufs) as wp,
            tc.tile_pool(name="big", bufs=big_bufs) as bp,
            tc.tile_pool(name="sm", bufs=sm_bufs) as sp,
            tc.tile_pool(name="psA", bufs=2, space="PSUM") as psA,
            tc.tile_pool(name="psM", bufs=1, space="PSUM") as psM,
        ):
            # ---- one-time constants (outside the timing loop) ----
            onesR = cp.tile([1, R], f32r, tag="onesR")
            onesRb = cp.tile([1, R], bf16, tag="onesRb")
            onesRf = cp.tile([1, R], f32, tag="onesRf")
            invL = cp.tile([PD, 1], bf16, tag="invL")
            invLnB = cp.tile([PD, PD], bf16, tag="invLnB")
            invLnBf = cp.tile([PD, PD], f32, tag="invLnBf")
            tmpf = cp.tile([PD, 1], f32, tag="tmpf")
            ones4 = cp.tile([E, 1], f32r, tag="ones4")
            ones4f = cp.tile([E, 1], f32, tag="ones4f")
            cbias = cp.tile([1, 1], f32, tag="cbias")
            zbias = cp.tile([PD, 1], f32, tag="zbias")
            gb4 = cp.tile([E, 1], f32, tag="gb4")
            nc.gpsimd.memset(onesRf, 1.0)
            nc.vector.tensor_copy(onesR, onesRf)
            nc.vector.tensor_copy(onesRb, onesRf)
            nc.gpsimd.memset(tmpf, 1.0 / L)
            nc.vector.tensor_copy(invL, tmpf)
            nc.gpsimd.memset(invLnBf, -1.0 / L)
            nc.vector.tensor_copy(invLnB, invLnBf)
            nc.gpsimd.memset(ones4f, 1.0)
            nc.vector.tensor_copy(ones4, ones4f)
            nc.gpsimd.memset(cbias, EPS)
            nc.vector.memset(zbias, 0.0)
            ones_r = onesR[:, 0:PD]
            sel = cp.tile([E, SELC], f32r, tag="sel")
            nc.scalar.dma_start(out=sel, in_=sel_d[:])
            # gate bias column as f32 for the ACT bias port
            nc.vector.tensor_copy(gb4, sel[:, E * P : E * P + 1].bitcast(f32))
            ld = mybir.InstLoadActFuncSet(
                name=nc.get_next_instruction_name(), ins=[], outs=[]
            )
            ld.act_func_set_id = actset
            ld.engine = nc.scalar.engine
            nc.scalar.add_instruction(ld)

            outer = (
                tc.For_i(0, loop_n, 1, staggered_reset=STAGGER)
                if loop_n > 1
                else nullcontext()
            )
            with outer:
                for _ in range(unroll):
                    # ---- DMAs ----
                    xs = bp.tile([PD, NCH * R], bf16, tag="xs")
                    wsb = wp.tile([PD, WAUX], bf16, tag="wsb")
                    H = NCH * R // 2
                    nc.sync.dma_start(out=xs[:, 0:H], in_=x_d[:, 0:H])
                    nc.sync.dma_start(out=xs[:, H:], in_=x_d[:, H:])
                    nc.sync.dma_start(out=wsb, in_=w_d[:])
                    aux = wsb[0:1, WTOT : WTOT + E * P]

                    c10 = wsb[:, 0 * WCOLS : 1 * WCOLS]
                    c20 = wsb[:, 1 * WCOLS : 2 * WCOLS]
                    c11 = wsb[:, 2 * WCOLS : 3 * WCOLS]
                    c21 = wsb[:, 3 * WCOLS : 4 * WCOLS]
                    ww0 = wsb[:, 4 * WCOLS : 5 * WCOLS]
                    ww1 = wsb[:, 5 * WCOLS : 6 * WCOLS]
                    wg = wsb[:, 6 * WCOLS : 6 * WCOLS + NCH * E]
                    bmm = [aux[:, e * P : (e + 1) * P] for e in range(E)]

                    # ---- PSUM banks ----
                    bankA = psA.tile([PD, 2 * R], f32, tag="bankA")
                    pacB = psA.tile([PD, 2 * R], f32, tag="pacB")
                    ptP = psM.tile([P, 2 * R], f32, tag="ptP")
                    pG01 = psM.tile([P, 2 * R], f32, tag="pG01")
                    pG23 = psM.tile([P, 2 * R], f32, tag="pG23")
                    pwP = psM.tile([P, 2 * R], f32, tag="pwP")
                    ex2_ps = bankA[0:1, R : 2 * R]
                    pg = bankA[0:E, R : 2 * R]        # after lnv reads ex2
                    prs = bankA[0:1, 0:R]
                    pacA = pacB[:, 0:R]
                    pacCM = pacB[:, R : 2 * R]
                    pt0 = ptP[:, 0:R]
                    pt1 = ptP[:, R:]
                    pw0 = pwP[:, 0:R]
                    pw1 = pwP[:, R:]

                    # ---- SBUF tiles ----
                    xsc = bp.tile([PD, NCH * R], bf16, tag="xsc")
                    sqc = bp.tile([PD, NCH * R], bf16, tag="sqc")
                    xn = bp.tile([PD, NCH * R], bf16, tag="xn")
                    x2n = bp.tile([PD, NCH * R], bf16, tag="x2n")
                    eT = bp.tile([PD, NCH * R], bf16, tag="eT")
                    psib = bp.tile([PD, NCH * R], bf16, tag="psib")
                    lnv = sp.tile([1, R], f32r, tag="lnv")
                    stdr = sp.tile([1, R], f32, tag="stdr")
                    rinv = sp.tile([1, R], f32, tag="rinv")
                    dmrow = sp.tile([1, R], f32r, tag="dmrow")
                    expg = sp.tile([E, R], f32r, tag="expg")
                    CMsb = bp.tile([PD, R], bf16, tag="CMsb")
                    Asb = bp.tile([PD, R], bf16, tag="Asb")
                    A2sb = bp.tile([PD, R], bf16, tag="A2sb")
                    ptsb = bp.tile([P, 2 * R], bf16, tag="ptsb")
                    DAsb = bp.tile([P, R], bf16, tag="DAsb")
                    G23sb = bp.tile([P, 2 * R], bf16, tag="G23sb")
                    m01 = bp.tile([P, 2 * R], bf16, tag="m01")
                    m23 = bp.tile([P, 2 * R], bf16, tag="m23")
                    s1 = bp.tile([P, R], bf16, tag="s1")
                    s2 = bp.tile([P, R], bf16, tag="s2")
                    s12 = bp.tile([P, R], bf16, tag="s12")
                    outp = bp.tile([P, R], bf16, tag="outp")

                    def CS(c):
                        return slice(c * R, (c + 1) * R)

                    def CP(c):
                        return slice(c * P, (c + 1) * P)

                    # ---- stats: -mean broadcast directly from the PE ----
                    for c in range(NCH):
                        nc.tensor.matmul(
                            pacCM, invLnB, xs[:, CS(c)],
                            start=(c == 0), stop=(c == NCH - 1),
                        )
                    nc.scalar.activation(CMsb, pacCM, AF.Copy)

                    # ---- xsc = x + CM ; sqc = xsc^2 ; var matmuls ----
                    for c in range(NCH):
                        e = eng(st["xsc"][c])
                        src = pacCM if c == 0 else CMsb
                        e.tensor_add(xsc[:, CS(c)], xs[:, CS(c)], src)
                    for c in range(NCH):
                        en = st["sqc"][c]
                        if en == "act":
                            nc.scalar.activation(
                                sqc[:, CS(c)], xsc[:, CS(c)], AF.Square, bias=zbias
                            )
                        else:
                            eng(en).tensor_mul(
                                sqc[:, CS(c)], xsc[:, CS(c)], xsc[:, CS(c)]
                            )
                        nc.tensor.matmul(
                            ex2_ps, invL, sqc[:, CS(c)],
                            start=(c == 0), stop=(c == NCH - 1),
                        )

                    # ---- var -> ln -> broadcast -> A (istd), A2 (istd^2) ----
                    nc.scalar.activation(lnv, ex2_ps, AF.Ln, bias=cbias)
                    nc.tensor.matmul(pacA, ones_r, lnv, start=True, stop=True)
                    nc.scalar.activation(Asb, pacA, AF.Exp, bias=zbias, scale=-0.5)
                    nc.scalar.activation(A2sb, pacA, AF.Exp, bias=zbias, scale=-1.0)
                    nc.scalar.activation(stdr, lnv, AF.Exp, bias=zbias[0:1], scale=0.5)

                    # ---- taylor bias matmuls OPEN the pt group. The wave
                    # bias matmuls are emitted just before the wave loop:
                    # placed here they would wait on pwP's previous-iteration
                    # reader (m23) at the HEAD of the PE FIFO and stall every
                    # gate/taylor matmul behind them. ----
                    nc.tensor.matmul(pt0, bmm[0], onesRb, start=True, stop=False)
                    nc.tensor.matmul(pt1, bmm[1], onesRb, start=False, stop=False)

                    # ---- xn / x2n / eT / psi elementwise + gate/taylor
                    # matmuls. Wave matmuls are emitted in a second loop so
                    # they never head-of-line-block the taylor close. ----
                    for c in range(NCH):
                        eng(st["xn"][c]).tensor_mul(
                            xn[:, CS(c)], xsc[:, CS(c)], Asb
                        )
                        eng(st["x2n"][c]).tensor_mul(
                            x2n[:, CS(c)], sqc[:, CS(c)], A2sb
                        )
                        last = c == NCH - 1
                        nc.tensor.matmul(
                            pg, wg[:, c * E : (c + 1) * E], xn[:, CS(c)],
                            start=(c == 0), stop=last,
                        )
                        nc.tensor.matmul(pt0, c10[:, CP(c)], xn[:, CS(c)],
                                         start=False, stop=False)
                        nc.tensor.matmul(pt1, c11[:, CP(c)], xn[:, CS(c)],
                                         start=False, stop=False)
                        if last:
                            nc.scalar.activation(expg, pg, AF.Exp, bias=gb4)
                        nc.tensor.matmul(pt0, c20[:, CP(c)], x2n[:, CS(c)],
                                         start=False, stop=False)
                        nc.tensor.matmul(pt1, c21[:, CP(c)], x2n[:, CS(c)],
                                         start=False, stop=last)
                        nc.scalar.activation(eT[:, CS(c)], x2n[:, CS(c)], AF.Exp,
                                             bias=zbias, scale=-0.5)
                        eng(st["psi"][c]).scalar_tensor_tensor(
                            psib[:, CS(c)], x2n[:, CS(c)], -1.0, eT[:, CS(c)],
                            op0=OP.add, op1=OP.mult,
                        )
                        if last:
                            nc.tensor.matmul(prs, ones4, expg, start=True, stop=True)

                    # taylor PSUM -> SBUF as soon as its group closes
                    if st["ptc"] == "act":
                        nc.scalar.activation(ptsb, ptP, AF.Copy)
                    else:
                        eng(st["ptc"]).tensor_copy(ptsb, ptP)

                    # wave matmuls (terminal chain); bias mms open the group
                    nc.tensor.matmul(pw0, bmm[2], onesRb, start=True, stop=False)
                    nc.tensor.matmul(pw1, bmm[3], onesRb, start=False, stop=False)
                    for c in range(NCH):
                        nc.tensor.matmul(pw0, ww0[:, CP(c)], psib[:, CS(c)],
                                         start=False, stop=False)
                        nc.tensor.matmul(pw1, ww1[:, CP(c)], psib[:, CS(c)],
                                         start=False, stop=(c == NCH - 1))

                    # ---- G planes bcast raw scores immediately; the denorm
                    # scale runs in parallel and applies once at the end ----
                    for e in range(2):
                        nc.tensor.matmul(
                            pG01[:, e * R : (e + 1) * R],
                            sel[:, e * P : (e + 1) * P],
                            expg, start=True, stop=True,
                        )
                        nc.tensor.matmul(
                            pG23[:, e * R : (e + 1) * R],
                            sel[:, (e + 2) * P : (e + 3) * P],
                            expg, start=True, stop=True,
                        )
                    # wave-side score plane to SBUF (overlaps the wave tail)
                    if st["pwc"] == "act":
                        nc.scalar.activation(G23sb, pG23, AF.Copy)
                    else:
                        eng(st["pwc"]).tensor_copy(G23sb, pG23)
                    # denorm row: stdev/(sum expg), broadcast over P partitions
                    nc.vector.reciprocal(rinv, prs)
                    nc.vector.tensor_mul(dmrow, stdr, rinv)
                    pdadc = pacB[0:P, 0:R]   # pacA region, free after Asb/A2sb
                    nc.tensor.matmul(pdadc, ones_r[:, 0:P], dmrow, start=True, stop=True)
                    nc.scalar.activation(DAsb, pdadc, AF.Copy)

                    # ---- mixture + denorm: taylor side = SBUF copy x PSUM
                    # plane; wave side = PSUM accum x SBUF plane ----
                    nc.vector.tensor_mul(m01, ptsb, pG01)
                    nc.vector.tensor_add(s1, m01[:, 0:R], m01[:, R:])
                    eng(st["m23"]).tensor_mul(m23, pwP, G23sb)
                    nc.vector.tensor_add(s2, m23[:, 0:R], m23[:, R:])
                    eng(st["s12"]).tensor_add(s12, s1, s2)
                    nc.vector.tensor_mul(s12, s12, DAsb)
                    eng(st["outp"]).tensor_sub(outp, s12, CMsb[0:P])
                    nc.scalar.dma_start(out=out_d[:], in_=outp)

    nc.compile()
    return nc


def _chunked(wT):
    """[L, M] -> [128, NCH*M], column block c holds rows l = c*128..(c+1)*128."""
    Lx, M = wT.shape
    return np.ascontiguousarray(
        wT.reshape(NCH, PD, M).transpose(1, 0, 2).reshape(PD, NCH * M)
    )


def _host_prep(inputs):
    import ml_dtypes

    f = np.float32
    bf = ml_dtypes.bfloat16
    g = {k: np.asarray(v, f) for k, v in inputs.items()}

    bn_scale = MH / math.sqrt(1.0 + BN_EPS)
    wparts = [
        _chunked(np.ascontiguousarray(g["t0_coeffs"][:, :, 1].T)),
        _chunked(np.ascontiguousarray(g["t0_coeffs"][:, :, 2].T)),
        _chunked(np.ascontiguousarray(g["t1_coeffs"][:, :, 1].T)),
        _chunked(np.ascontiguousarray(g["t1_coeffs"][:, :, 2].T)),
        _chunked(
            np.ascontiguousarray((g["w0_ww"] * g["w0_gamma"][:, None] * bn_scale).T)
        ),
        _chunked(
            np.ascontiguousarray((g["w1_ww"] * g["w1_gamma"][:, None] * bn_scale).T)
        ),
        _chunked(np.ascontiguousarray(g["gate_w"].T)),
    ]
    w_h = np.concatenate(wparts, axis=1).astype(bf)
    assert w_h.shape == (PD, WTOT)

    aux_h = np.zeros((1, E * P + E), f)
    aux_h[0, 0:P] = (
        g["t0_coeffs"][:, :, 0].sum(axis=1, dtype=np.float64) + g["t0_bias"][0]
    ).astype(f)
    aux_h[0, P : 2 * P] = (
        g["t1_coeffs"][:, :, 0].sum(axis=1, dtype=np.float64) + g["t1_bias"][0]
    ).astype(f)
    aux_h[0, 2 * P : 3 * P] = g["w0_beta"]
    aux_h[0, 3 * P : 4 * P] = g["w1_beta"]
    wa_h = np.zeros((PD, E * P + E), f)
    wa_h[0, :] = aux_h[0]
    w_h = np.concatenate([w_h, wa_h.astype(bf)], axis=1)
    assert w_h.shape == (PD, WAUX)

    sel_h = np.zeros((E, SELC), f)
    for e in range(E):
        sel_h[e, e * P : (e + 1) * P] = 1.0
    sel_h[:, E * P] = g["gate_b"] + np.float32(math.log1p(EPS))
    common = {"w": w_h, "sel": sel_h}

    x = g["x"]
    xcores = []
    for i in range(NCORES):
        xc = x[i * BPC : (i + 1) * BPC]  # [BPC, L, N]
        xcores.append(
            np.ascontiguousarray(
                xc.reshape(BPC, NCH, PD, N).transpose(2, 1, 0, 3).reshape(PD, NCH * R)
            ).astype(bf)
        )
    return common, xcores


def _fast_ok(inputs):
    try:
        return (
            np.all(np.asarray(inputs["w0_scale"]) == 1.0)
            and np.all(np.asarray(inputs["w1_scale"]) == 1.0)
            and np.all(np.asarray(inputs["w0_trans"]) == 0.0)
            and np.all(np.asarray(inputs["w1_trans"]) == 0.0)
            and np.all(np.asarray(inputs["rev_w"]) == 1.0)
            and np.all(np.asarray(inputs["rev_b"]) == 0.0)
        )
    except Exception:
        return False


def _numpy_ref(inputs):
    """Exact general fallback (host numpy), mirrors the reference module."""
    g = {k: np.asarray(v, np.float32) for k, v in inputs.items()}
    x = g["x"]
    mean = x.mean(axis=1, keepdims=True)
    stdev = np.sqrt(x.var(axis=1, keepdims=True) + np.float32(EPS))
    xn = (x - mean) / stdev * g["rev_w"] + g["rev_b"]
    xf = xn.transpose(0, 2, 1).reshape(B * N, L)
    logits = xf @ g["gate_w"].T + g["gate_b"]
    logits -= logits.max(axis=-1, keepdims=True)
    elg = np.exp(logits)
    score = elg / elg.sum(axis=-1, keepdims=True)

    def taylor(c, b):
        y = np.full((B * N, P), c[:, :, 0].sum(axis=1), np.float32)
        y += xf @ c[:, :, 1].T + (xf * xf) @ c[:, :, 2].T
        return y + b

    def wave(s, t, w, gam, bet):
        y = np.empty((B * N, P), np.float32)
        for i0 in range(0, B * N, 128):
            xs = (xf[i0 : i0 + 128, None, :] - t[None]) / s[None]
            x2 = xs * xs
            psi = np.float32(MH) * (x2 - 1.0) * np.exp(-0.5 * x2)
            y[i0 : i0 + 128] = np.einsum("bpl,pl->bp", psi, w)
        return (y / np.sqrt(np.float32(1.0 + BN_EPS))) * gam + bet

    eo = np.stack(
        [
            taylor(g["t0_coeffs"], g["t0_bias"][0]),
            taylor(g["t1_coeffs"], g["t1_bias"][0]),
            wave(g["w0_scale"], g["w0_trans"], g["w0_ww"], g["w0_gamma"], g["w0_beta"]),
            wave(g["w1_scale"], g["w1_trans"], g["w1_ww"], g["w1_gamma"], g["w1_beta"]),
        ],
        axis=-1,
    )
    pred = np.einsum("bpE,bE->bp", eo, score)
    pred = pred.reshape(B, N, P).transpose(0, 2, 1)
    out = ((pred - g["rev_b"]) / (g["rev_w"] + np.float32(EPS))) * stdev + mean
    return out.astype(np.float32)


def run(inputs, trace=False):
    """Run the Bass kernel on 8 cores. Returns (out [B,P,N], exec_time_ns|None)."""
    from concourse.bass_utils import run_bass_kernel_spmd

    if "nc" not in _NC_CACHE:
        _NC_CACHE["nc"] = _build_nc()
    nc = _NC_CACHE["nc"]
    common, xcores = _host_prep(inputs)
    in_maps = [dict(common, x=xcores[i]) for i in range(NCORES)]
    try:
        res = run_bass_kernel_spmd(nc, in_maps, list(range(NCORES)), trace=trace)
    except ModuleNotFoundError:
        res = run_bass_kernel_spmd(nc, in_maps, list(range(NCORES)), trace=False)
    out = np.empty((B, P, N), np.float32)
    for i in range(NCORES):
        o = np.asarray(res.results[i]["out"]).astype(np.float32).reshape(P, BPC, N)
        out[i * BPC : (i + 1) * BPC] = o.transpose(1, 0, 2)
    return out, res.exec_time_ns


def kernel(**inputs):
    if not _fast_ok(inputs):
        return _numpy_ref(inputs)
    out, _ = run(inputs)
    return out
